# revision 22
# baseline (speedup 1.0000x reference)
"""Trainium2 Bass kernel for a GPT-2 style transformer block, 8-core SPMD.

Sharding: core c handles sequence c//2, query-parity c%2. Each core owns the
8 interleaved 128-token query blocks (even core: blocks 0,2,..,14; odd core:
blocks 1,3,..,15) of its sequence, computes full k/v for that sequence, runs
causal attention for its query blocks (all 16 heads), then the MLP for its
1024 tokens. Outputs are scattered back on the host. No collectives; the
per-core programs are instruction-identical (SPMD) — causality differences
are encoded in per-core mask *data* (multiplicative {0,1} masks on the last
two key-tiles of each query block).

Merged projection+attention ("AB") phase: v is projected up front, then per
head-pair hp: project q(hp), k(hp), then run attention for hp over all query
blocks. The softmax Exp chain (activation engine, the attention bottleneck)
overlaps the next head-pair's projection matmuls, keeping both the PE and ACT
engines busy.

Attention layout: scores are computed transposed (sT[key, q] = k . q) so that
both PV operands are keys-on-partition and softmax needs no transposes; the
denominator comes from an appended ones-column in V (out = [av | sum(p)]),
and the per-row normalization happens token-major where it is a cheap
per-partition scalar multiply. The two heads of a pair occupy partition
halves 0-63 / 64-127; their score matmuls are issued adjacently so the PE
runs them concurrently on disjoint row-groups (tile_position row tiling).

PSUM discipline: start=True marks the whole 2KB zero-region pending, so
every concurrently-accumulating group needs its own bank. The two PV
accumulators (per head) process the Left query block first, then reuse the
same banks for the Right block (the exp'd probability tiles persist in SBUF).

Matmul inputs are bf16 (fp32 PSUM accumulation); residual/layernorm fp32.
"""

import sys
from contextlib import ExitStack

import numpy as np

for _p in ("/opt/trn_rl_repo",):
    if _p not in sys.path:
        sys.path.insert(0, _p)

import ml_dtypes

import concourse.bass as bass
import concourse.mybir as mybir
import concourse.tile as tile
from concourse import bacc
from concourse.bass_utils import run_bass_kernel_spmd
from concourse.masks import make_identity

BF16 = ml_dtypes.bfloat16
F32 = mybir.dt.float32
BF = mybir.dt.bfloat16
P = 128
AF = mybir.ActivationFunctionType
ALU = mybir.AluOpType


def chunks(total, size=512):
    return [(s, min(size, total - s)) for s in range(0, total, size)]


class Cfg:
    """Problem geometry. Defaults = the real problem; overridable for sims."""

    def __init__(self, S=2048, D=1024, H=16, DFF=4096):
        self.S = S
        self.D = D
        self.H = H
        self.DFF = DFF
        self.HD = 64
        assert self.H * self.HD == self.D
        self.NQ = S // 2
        self.NSLOT = S // 256
        self.KT = D // P
        self.MT_FF = DFF // P
        self.NQ_T = self.NQ // P
        self.S_T = S // P


def build_nc(cfg: Cfg):
    S, D, H, DFF = cfg.S, cfg.D, cfg.H, cfg.DFF
    NQ, NSLOT, KT, MT_FF = cfg.NQ, cfg.NSLOT, cfg.KT, cfg.MT_FF
    NQ_T, S_T = cfg.NQ_T, cfg.S_T
    NPAIR = H // 2

    nc = bacc.Bacc(None, target_bir_lowering=False, debug=False)

    def din(name, shape, d=BF):
        return nc.dram_tensor(name, shape, d, kind="ExternalInput").ap()

    xt = din("xt", [D, S])                 # x[b].T, full sequence
    xtq = din("xtq", [P, KT * NQ])         # own tokens, [p, kt*NQ+t] layout
    xq = din("xq", [NQ, D], F32)           # own tokens, token-major (residual)
    masks = din("masks", [4, P, 2 * P])    # multiplicative causal masks (bf16)
    wqkv = din("wqkv", [D, 3 * D])         # q-part pre-scaled by 1/8
    bqkv = din("bqkv", [P, 2 * KT], F32)   # q,k bias packed (col%128, col//128)
    bvrow = din("bvrow", [1, D], F32)      # v bias as a row
    wap = din("wap", [D, D])
    bap = din("bap", [1, D], F32)
    wfc = din("wfc", [D, DFF])
    bfc = din("bfc", [P, MT_FF], F32)
    wmp = din("wmp", [DFF, D])
    bmp = din("bmp", [1, D], F32)
    g1 = din("g1", [1, D], F32)
    g2 = din("g2", [1, D], F32)
    b2 = din("b2", [1, D], F32)
    out = nc.dram_tensor("out", [NQ, D], F32, kind="ExternalOutput").ap()

    wqkv3 = wqkv.rearrange("(kt p) c -> kt p c", p=P)
    wap3 = wap.rearrange("(kt p) c -> kt p c", p=P)
    wfc3 = wfc.rearrange("(kt p) c -> kt p c", p=P)
    wmp3 = wmp.rearrange("(kt p) c -> kt p c", p=P)
    xtq3 = xtq.rearrange("p (kt t) -> p kt t", kt=KT)

    def bcast_row(ap):  # [1, D] DRAM -> [P, D] broadcast AP
        return bass.AP(tensor=ap.tensor, offset=ap.offset, ap=[[0, P], [1, D]])

    NSUB = (D + 511) // 512
    SUB = D // NSUB
    assert SUB * NSUB == D and SUB <= 512

    def layer_norm(nc, pool, dst, src, g_row, b_row, eps_t, tag, rows=None):
        """dst[P, D] = g * (src - mean)/sqrt(var+eps) + b, rowwise over D.
        rows=False skips the g/b application entirely."""
        skip_rows = rows is False
        rows = nc.gpsimd if rows in (None, False) else rows
        stats = pool.tile([P, NSUB, 6], F32, tag=f"{tag}_st")
        for sub in range(NSUB):
            nc.vector.bn_stats(stats[:, sub, :], src[:, sub * SUB:(sub + 1) * SUB])
        mv = pool.tile([P, 2], F32, tag=f"{tag}_mv")
        nc.vector.bn_aggr(mv, stats)
        rstd = pool.tile([P, 1], F32, tag=f"{tag}_rs")
        nc.scalar.activation(out=rstd, in_=mv[:, 1:2], func=AF.Sqrt,
                             bias=eps_t, scale=1.0)
        nc.vector.reciprocal(rstd, rstd)
        nc.vector.tensor_scalar(
            out=dst, in0=src, scalar1=mv[:, 0:1], scalar2=rstd,
            op0=ALU.subtract, op1=ALU.mult)
        if not skip_rows:
            rows.tensor_mul(dst, dst, g_row)
            rows.tensor_add(dst, dst, b_row)

    with tile.TileContext(nc) as tc, ExitStack() as top:
        const = top.enter_context(tc.tile_pool(name="const", bufs=1))
        bqkvS = const.tile([P, 2 * KT], F32)
        nc.sync.dma_start(out=bqkvS, in_=bqkv)
        epsS = const.tile([P, 1], F32)
        nc.vector.memset(epsS, 1e-5)
        bfcS = const.tile([P, MT_FF], F32)
        nc.sync.dma_start(out=bfcS, in_=bfc)

        npool = top.enter_context(tc.tile_pool(name="npool", bufs=1))
        NHALF = max(1, NQ // 512)

        with ExitStack() as s_a:
            preh = s_a.enter_context(tc.tile_pool(name="pre_xt", bufs=1))
            aper = s_a.enter_context(tc.tile_pool(name="aper", bufs=1))
            aSB = [aper.tile([P, D], BF, name=f"aSB{j}", tag=f"aSB{j}")
                   for j in range(NQ_T)]
            maskS = aper.tile([P, 4, 2 * P], BF)
            nc.sync.dma_start(out=maskS, in_=masks.rearrange("m k q -> k m q"))

            with ExitStack() as s_qkv:
                qkvp = s_qkv.enter_context(tc.tile_pool(name="qkvper", bufs=1))
                vA = qkvp.tile([P, S_T, H, 65], BF)
                abp = s_qkv.enter_context(tc.tile_pool(name="abp", bufs=1))
                xtqS = abp.tile([P, KT, NQ], BF)
                qtp = s_qkv.enter_context(tc.tile_pool(name="qtp", bufs=2))
                ktp = s_qkv.enter_context(tc.tile_pool(name="ktp", bufs=2))
                wp = s_qkv.enter_context(tc.tile_pool(name="wp", bufs=2))
                wvp = s_qkv.enter_context(tc.tile_pool(name="wvp", bufs=1))
                ptp = s_qkv.enter_context(tc.tile_pool(name="ptp", bufs=8))
                pm = s_qkv.enter_context(tc.tile_pool(name="pm", bufs=4))
                pqk = s_qkv.enter_context(
                    tc.tile_pool(name="pqk", bufs=2, space="PSUM"))
                psp = s_qkv.enter_context(
                    tc.tile_pool(name="psp", bufs=2, space="PSUM"))
                pvp = s_qkv.enter_context(
                    tc.tile_pool(name="pvp", bufs=1, space="PSUM"))

                # ---- input staging ----
                xtS = preh.tile([P, KT, S], BF, tag="bigslot")
                for h0, hw in chunks(S, S // 2):
                    nc.gpsimd.dma_start(
                        out=xtS[:, :, h0:h0 + hw],
                        in_=xt.rearrange("(kt p) t -> p kt t", p=P)
                        [:, :, h0:h0 + hw])
                for h0, hw in chunks(NQ, NQ // 2):
                    nc.sync.dma_start(out=xtqS[:, :, h0:h0 + hw],
                                      in_=xtq3[:, :, h0:h0 + hw])

                def xt_slice(kt, a, b):
                    return xtS[:, kt, a:b]

                # ---- v projection (full sequence, all heads) ----
                with ExitStack() as ph:
                    bvp = ph.enter_context(tc.tile_pool(name="bvp", bufs=1))
                    bvS = bvp.tile([P, D], F32)
                    nc.sync.dma_start(out=bvS, in_=bcast_row(bvrow))
                    for c0, cw in chunks(D):
                        wv = wvp.tile([P, KT, 512], BF, tag="wv")
                        nc.sync.dma_start(
                            out=wv[:, :, :cw],
                            in_=wqkv3[:, :, 2 * D + c0:2 * D + c0 + cw]
                            .rearrange("kt p c -> p kt c"))
                        h0, nh = c0 // 64, cw // 64
                        for mt in range(S_T):
                            ps = pqk.tile([P, 512], F32, tag="qk")
                            for kt in range(KT):
                                nc.tensor.matmul(
                                    ps[:, :cw],
                                    lhsT=xt_slice(kt, mt * P, (mt + 1) * P),
                                    rhs=wv[:, kt, :cw],
                                    start=(kt == 0), stop=(kt == KT - 1))
                            nc.vector.tensor_add(
                                out=vA[:, mt, h0:h0 + nh, 0:64],
                                in0=ps[:, :cw]
                                .rearrange("p (h d) -> p h d", d=64),
                                in1=bvS[:, c0:c0 + cw]
                                .rearrange("p (h d) -> p h d", d=64))
                    nc.vector.memset(vA[:, :, :, 64:65], 1.0)

                # ---- per head-pair: q/k projection then attention ----
                for hp in range(NPAIR):
                    wq = wp.tile([P, KT, P], BF, tag="wq")
                    nc.sync.dma_start(
                        out=wq, in_=wqkv3[:, :, hp * P:(hp + 1) * P]
                        .rearrange("kt p c -> p kt c"))
                    qTt = qtp.tile([P, NQ], BF, tag="qT")
                    for c0, w in chunks(NQ):
                        ps = pqk.tile([P, 512], F32, tag="qk")
                        for kt in range(KT):
                            nc.tensor.matmul(
                                ps[:, :w], lhsT=wq[:, kt, :],
                                rhs=xtqS[:, kt, c0:c0 + w],
                                start=(kt == 0), stop=(kt == KT - 1))
                        nc.vector.tensor_scalar_add(
                            out=qTt[:, c0:c0 + w], in0=ps[:, :w],
                            scalar1=bqkvS[:, hp:hp + 1])

                    wk = wp.tile([P, KT, P], BF, tag="wk")
                    nc.sync.dma_start(
                        out=wk, in_=wqkv3[:, :, D + hp * P:D + (hp + 1) * P]
                        .rearrange("kt p c -> p kt c"))
                    kTt = ktp.tile([P, S], BF, tag="kT")
                    for c0, w in chunks(S):
                        ps = pqk.tile([P, 512], F32, tag="qk")
                        for kt in range(KT):
                            nc.tensor.matmul(
                                ps[:, :w], lhsT=wk[:, kt, :],
                                rhs=xt_slice(kt, c0, c0 + w),
                                start=(kt == 0), stop=(kt == KT - 1))
                        nc.vector.tensor_scalar_add(
                            out=kTt[:, c0:c0 + w], in0=ps[:, :w],
                            scalar1=bqkvS[:, KT + hp:KT + hp + 1])

                    for g in range(NSLOT // 2):
                        nkt = 4 * g + 4
                        # Left query block: accumulate while scores stream
                        pavL = [pvp.tile([P, 65], F32, tag=f"pav{hh}",
                                         name=f"pavL{hh}") for hh in range(2)]
                        pts = []
                        for sc in range(nkt // 2):
                            lo = 2 * sc
                            ps = psp.tile([P, 2, 2, 2 * P], F32, tag="ps")
                            for ki in range(2):
                                kt = lo + ki
                                for hh in range(2):
                                    pb = 64 * hh
                                    nc.tensor.matmul(
                                        ps[:, hh, ki, :],
                                        lhsT=kTt[pb:pb + 64,
                                                 kt * P:(kt + 1) * P],
                                        rhs=qTt[pb:pb + 64,
                                                g * 2 * P:(g + 1) * 2 * P],
                                        start=True, stop=True)
                            pt = ptp.tile([P, 2, 2, 2 * P], BF, tag="pt")
                            nc.scalar.activation(out=pt, in_=ps, func=AF.Exp)
                            if lo >= nkt - 4:  # masks cover last 4 k-tiles
                                m0 = lo - (nkt - 4)
                                for hh in range(2):
                                    nc.vector.tensor_mul(
                                        pt[:, hh, :, :], pt[:, hh, :, :],
                                        maskS[:, m0:m0 + 2, :])
                            pts.append((lo, pt))
                            for hh in range(2):
                                h = 2 * hp + hh
                                for ki in range(2):
                                    kt = lo + ki
                                    if kt < nkt - 2:
                                        nc.tensor.matmul(
                                            pavL[hh],
                                            lhsT=pt[:, hh, ki, 0:P],
                                            rhs=vA[:, kt, h, :],
                                            start=(kt == 0),
                                            stop=(kt == nkt - 3))
                        for hh in range(2):
                            h = 2 * hp + hh
                            rec = pm.tile([P, 1], F32, tag="rec")
                            nc.vector.reciprocal(rec, pavL[hh][:, 64:65])
                            nc.vector.tensor_scalar_mul(
                                out=aSB[2 * g][:, h * 64:(h + 1) * 64],
                                in0=pavL[hh][:, 0:64], scalar1=rec)
                        # Right query block: replay persisted pt tiles into
                        # the same two banks (generation WAR on the drain)
                        pavR = [pvp.tile([P, 65], F32, tag=f"pav{hh}",
                                         name=f"pavR{hh}") for hh in range(2)]
                        for lo, pt in pts:
                            for hh in range(2):
                                h = 2 * hp + hh
                                for ki in range(2):
                                    kt = lo + ki
                                    nc.tensor.matmul(
                                        pavR[hh],
                                        lhsT=pt[:, hh, ki, P:2 * P],
                                        rhs=vA[:, kt, h, :],
                                        start=(kt == 0),
                                        stop=(kt == nkt - 1))
                        for hh in range(2):
                            h = 2 * hp + hh
                            rec = pm.tile([P, 1], F32, tag="rec")
                            nc.vector.reciprocal(rec, pavR[hh][:, 64:65])
                            nc.vector.tensor_scalar_mul(
                                out=aSB[2 * g + 1][:, h * 64:(h + 1) * 64],
                                in0=pavR[hh][:, 0:64], scalar1=rec)

                # prefetch aproj weights into the (now dead) xtS slot; the
                # DMA transfers while the attention tail drains
                wapS = preh.tile([P, KT, D], BF, tag="bigslot", name="wapS")
                nc.sync.dma_start(
                    out=wapS, in_=wap3.rearrange("kt p c -> p kt c"))
            # per-hp tiles freed here

            # ---------- Phase C: aproj + residual + LN1 (+ transposes) ----
            with ExitStack() as ph:
                pcc = ph.enter_context(tc.tile_pool(name="pcc", bufs=1))
                ppc = ph.enter_context(
                    tc.tile_pool(name="pc_ps", bufs=3, space="PSUM"))
                pmc = ph.enter_context(tc.tile_pool(name="pc_m", bufs=3))
                ptr = ph.enter_context(
                    tc.tile_pool(name="pc_tr", bufs=2, space="PSUM"))

                idb = pcc.tile([P, P], BF)
                make_identity(nc, idb)
                aT = [pcc.tile([P, NQ], BF, name=f"aT{ck}", tag=f"aT{ck}")
                      for ck in range(KT)]
                n_sb = [npool.tile([P, D], BF, name=f"n{mt}", tag=f"n{mt}")
                        for mt in range(NQ_T)]
                nT = [[npool.tile([P, min(512, NQ)], BF, name=f"nT{ck}_{hf}",
                                  tag=f"nT{ck}_{hf}") for hf in range(NHALF)]
                      for ck in range(KT)]
                bapS = pmc.tile([P, D], F32, tag="bapS", bufs=1)
                nc.sync.dma_start(out=bapS, in_=bcast_row(bap))
                g1S = pmc.tile([P, D], F32, tag="g1S", bufs=1)
                nc.sync.dma_start(out=g1S, in_=bcast_row(g1))
                bmpS = pmc.tile([P, D], F32, tag="bmpS", bufs=1)
                nc.sync.dma_start(out=bmpS, in_=bcast_row(bmp))

                def transpose_a(g):
                    # aSB[2g], aSB[2g+1] (token-major) -> aT (feature-major)
                    for j in (2 * g, 2 * g + 1):
                        for ck in range(KT):
                            pst = ptr.tile([P, P], BF, tag="pst")
                            nc.tensor.transpose(
                                pst, aSB[j][:, ck * P:(ck + 1) * P], idb)
                            nc.vector.tensor_copy(
                                out=aT[ck][:, j * P:(j + 1) * P], in_=pst)

                xq_pref = {}

                def xq_tile(mt):
                    if mt < NQ_T and mt not in xq_pref:
                        t = pmc.tile([P, D], F32, tag="xqS", name=f"xq{mt}")
                        nc.sync.dma_start(
                            out=t, in_=xq[mt * P:(mt + 1) * P, :])
                        xq_pref[mt] = t

                xq_tile(0)
                xq_tile(1)
                for mt in range(NQ_T):
                    if mt % 2 == 0:
                        transpose_a(mt // 2)
                    xq_tile(mt + 2)
                    xqS = xq_pref.pop(mt)
                    for c0, w in chunks(D):
                        ps = ppc.tile([P, 512], F32, tag="ps")
                        for kt in range(KT):
                            nc.tensor.matmul(
                                ps[:, :w],
                                lhsT=aT[kt][:, mt * P:(mt + 1) * P],
                                rhs=wapS[:, kt, c0:c0 + w],
                                start=(kt == 0), stop=(kt == KT - 1))
                        sl = slice(c0, c0 + w)
                        nc.vector.scalar_tensor_tensor(
                            out=xqS[:, sl], in0=ps[:, :w], scalar=1.0,
                            in1=xqS[:, sl], op0=ALU.mult, op1=ALU.add)
                        nc.vector.tensor_add(xqS[:, sl], xqS[:, sl], bapS[:, sl])
                    layer_norm(nc, pmc, n_sb[mt], xqS, None, None,
                               epsS, "ln1", rows=False)

                mt_per_half = NQ_T // NHALF
                for mt in range(NQ_T):
                    hf, mo = mt // mt_per_half, mt % mt_per_half
                    for ck in range(KT):
                        pst = ptr.tile([P, P], BF, tag="pst")
                        nc.tensor.transpose(
                            pst, n_sb[mt][:, ck * P:(ck + 1) * P], idb)
                        nc.vector.tensor_copy(
                            out=nT[ck][hf][:, mo * P:(mo + 1) * P], in_=pst)
                # residual copy: n = g1*n0 + (b1 + b_mproj), off critical path
                for mt in range(NQ_T):
                    nc.gpsimd.tensor_mul(n_sb[mt], n_sb[mt], g1S)
                    nc.gpsimd.tensor_add(n_sb[mt], n_sb[mt], bmpS)
        # aSB/aT freed here

        # -------- Phase D: MLP + residual + LN2 --------
        with ExitStack() as ph:
            pd = ph.enter_context(tc.tile_pool(name="pd", bufs=1))
            pdw = ph.enter_context(tc.tile_pool(name="pd_w", bufs=4))
            pwm = ph.enter_context(tc.tile_pool(name="pd_wm", bufs=6))
            pmd = ph.enter_context(tc.tile_pool(name="pd_m", bufs=2))
            # one PSUM pool for all of phase D (no mid-phase pool barrier):
            # fc uses tag "ps" (2 banks); mproj alternates tags mpA/mpB
            # (2 banks each, one live accumulation group per bank)
            ppd = ph.enter_context(
                tc.tile_pool(name="pd_ps", bufs=1, space="PSUM"))

            g2S = pd.tile([P, D], F32)
            nc.sync.dma_start(out=g2S, in_=bcast_row(g2))
            b2S = pd.tile([P, D], F32)
            nc.sync.dma_start(out=b2S, in_=bcast_row(b2))

            # fc: all tokens in one pass (weights streamed once)
            hT = pd.tile([P, MT_FF, NQ], BF)
            for m in range(MT_FF):
                wfcT = pdw.tile([P, KT, P], BF, tag="wfc")
                nc.sync.dma_start(
                    out=wfcT, in_=wfc3[:, :, m * P:(m + 1) * P]
                    .rearrange("kt p c -> p kt c"))
                for hf in range(NHALF):
                    ps = ppd.tile([P, 512], F32, tag="ps", bufs=2)
                    for kt in range(KT):
                        nc.tensor.matmul(
                            ps, lhsT=wfcT[:, kt, :], rhs=nT[kt][hf],
                            start=(kt == 0), stop=(kt == KT - 1))
                    nc.scalar.activation(
                        out=hT[:, m, hf * 512:(hf + 1) * 512], in_=ps,
                        func=AF.Gelu_apprx_tanh, bias=bfcS[:, m:m + 1],
                        scale=1.0)

            # mproj in two token-halves so LN2/output of half 0 overlaps
            # half 1's matmuls. Each 512-col chunk accumulates 4 token tiles
            # at once (sharing one wm stream) across two 2-bank psum tiles
            # whose [:, ml, :] regions are bank-aligned (one group per bank).
            MH = 2
            mts_per_h = NQ_T // MH
            for mh in range(MH):
                mts = list(range(mh * mts_per_h, (mh + 1) * mts_per_h))
                r2s = {mt: pmd.tile([P, D], F32, tag=f"r2_{mt % 4}",
                                    name=f"r2_{mt}")
                       for mt in mts}
                for ci, (c0, w) in enumerate(chunks(D)):
                    pss = [ppd.tile([P, 2, 512], F32, tag=f"mp{half}",
                                    bufs=1, name=f"pss{mh}_{ci}_{half}")
                           for half in range(2)]
                    for kt in range(MT_FF):
                        wmT = pwm.tile([P, 512], BF, tag="wm")
                        nc.gpsimd.dma_start(out=wmT[:, :w],
                                            in_=wmp3[kt][:, c0:c0 + w])
                        for ml, mt in enumerate(mts):
                            nc.tensor.matmul(
                                pss[ml // 2][:, ml % 2, :w],
                                lhsT=hT[:, kt, mt * P:(mt + 1) * P],
                                rhs=wmT[:, :w],
                                start=(kt == 0), stop=(kt == MT_FF - 1))
                    sl = slice(c0, c0 + w)
                    for ml, mt in enumerate(mts):
                        nc.vector.scalar_tensor_tensor(
                            out=r2s[mt][:, sl], in0=pss[ml // 2][:, ml % 2, :w],
                            scalar=1.0, in1=n_sb[mt][:, sl],
                            op0=ALU.mult, op1=ALU.add)
                for mt in mts:
                    oS = pmd.tile([P, D], F32, tag="oS")
                    layer_norm(nc, pmd, oS, r2s[mt], g2S, b2S, epsS,
                               "ln2", rows=nc.vector)
                    nc.scalar.dma_start(
                        out=out[mt * P:(mt + 1) * P, :], in_=oS)

    nc.finalize()
    return nc


def host_inputs(cfg: Cfg, x, w_attn, b_attn, w_aproj, b_aproj, g1, b1,
                w_fc, b_fc, w_mproj, b_mproj, g2, b2, n_cores=8):
    """Build the per-core input maps + output scatter indices."""
    S, D = cfg.S, cfg.D
    NSLOT, KT, MT_FF = cfg.NSLOT, cfg.KT, cfg.MT_FF
    NQ = cfg.NQ

    wqkv = np.ascontiguousarray(w_attn, np.float32).copy()
    wqkv[:, :D] *= 0.125
    wqkv = wqkv.astype(BF16)
    b_adj = np.asarray(b_attn, np.float32).copy()
    b_adj[:D] *= 0.125
    bqkv = np.ascontiguousarray(b_adj[:2 * D].reshape(2 * KT, P).T, np.float32)
    shared = dict(
        wqkv=wqkv,
        bqkv=bqkv,
        bvrow=np.ascontiguousarray(
            np.asarray(b_attn, np.float32)[2 * D:].reshape(1, D)),
        wap=np.asarray(w_aproj).astype(BF16),
        bap=np.asarray(b_aproj, np.float32).reshape(1, D),
        wfc=(np.asarray(w_fc, np.float32)
             * np.asarray(g1, np.float32)[:, None]).astype(BF16),
        bfc=np.ascontiguousarray(
            (np.asarray(b_fc, np.float32)
             + np.asarray(b1, np.float32) @ np.asarray(w_fc, np.float32))
            .reshape(MT_FF, P).T),
        wmp=np.asarray(w_mproj).astype(BF16),
        bmp=(np.asarray(b_mproj, np.float32)
             + np.asarray(b1, np.float32)).reshape(1, D),
        g1=np.asarray(g1, np.float32).reshape(1, D),
        g2=np.asarray(g2, np.float32).reshape(1, D),
        b2=np.asarray(b2, np.float32).reshape(1, D),
    )

    tri_kq = (np.arange(P)[:, None] <= np.arange(P)[None, :]).astype(np.float32)
    ones = np.ones((P, P), np.float32)
    zeros = np.zeros((P, P), np.float32)

    def m4(r):
        if r == 0:
            rows = [[tri_kq, ones], [zeros, ones], [zeros, tri_kq],
                    [zeros, zeros]]
        else:
            rows = [[ones, ones], [tri_kq, ones], [zeros, ones],
                    [zeros, tri_kq]]
        return np.stack([np.concatenate(rr, axis=1) for rr in rows])

    m_even, m_odd = m4(0), m4(1)

    in_maps = []
    idx_all = []
    for c in range(n_cores):
        b, r = c // 2, c % 2
        idx = (np.arange(NSLOT)[:, None] * 256 + 128 * r
               + np.arange(P)[None, :]).ravel()
        idx_all.append((b, idx))
        xb = np.asarray(x[b], np.float32)
        m = dict(shared)
        m["xt"] = np.ascontiguousarray(xb.T).astype(BF16)
        xtq_km = np.ascontiguousarray(xb[idx]).T.reshape(KT, P, NQ)
        m["xtq"] = np.ascontiguousarray(
            xtq_km.transpose(1, 0, 2).reshape(P, KT * NQ)).astype(BF16)
        m["xq"] = np.ascontiguousarray(xb[idx], np.float32)
        m["masks"] = (m_even if r == 0 else m_odd).astype(BF16)
        in_maps.append(m)
    return in_maps, idx_all


_CACHE = {}


def _get_nc(cfg: Cfg):
    key = (cfg.S, cfg.D, cfg.H, cfg.DFF)
    if key not in _CACHE:
        _CACHE[key] = build_nc(cfg)
    return _CACHE[key]


def kernel(x, w_attn, b_attn, w_aproj, b_aproj, g1, b1, w_fc, b_fc,
           w_mproj, b_mproj, g2, b2):
    cfg = Cfg()
    nc = _get_nc(cfg)
    in_maps, idx_all = host_inputs(cfg, x, w_attn, b_attn, w_aproj, b_aproj,
                                   g1, b1, w_fc, b_fc, w_mproj, b_mproj,
                                   g2, b2)
    res = run_bass_kernel_spmd(nc, in_maps, list(range(8)))
    B = x.shape[0]
    y = np.empty((B, cfg.S, cfg.D), np.float32)
    for c in range(8):
        b, idx = idx_all[c]
        y[b][idx] = res.results[c]["out"]
    return y


# revision 23
# speedup vs baseline: 1.1949x; 1.1949x over previous
"""Trainium2 Bass kernel for a GPT-2 style transformer block, 8-core SPMD.

Sharding: core c handles sequence c//2, query-parity c%2. Each core owns the
8 interleaved 128-token query blocks (even core: blocks 0,2,..,14; odd core:
blocks 1,3,..,15) of its sequence, computes full k/v for that sequence, runs
causal attention for its query blocks (all 16 heads), then the MLP for its
1024 tokens. Outputs are scattered back on the host. No collectives; the
per-core programs are instruction-identical (SPMD) — causality differences
are encoded in per-core mask *data* (multiplicative {0,1} masks on the last
two key-tiles of each query block).

Merged projection+attention ("AB") phase: v is projected up front, then per
head-pair hp: project q(hp), k(hp), then run attention for hp over all query
blocks. The softmax Exp chain (activation engine, the attention bottleneck)
overlaps the next head-pair's projection matmuls, keeping both the PE and ACT
engines busy.

Attention layout: scores are computed transposed (sT[key, q] = k . q) so that
both PV operands are keys-on-partition and softmax needs no transposes; the
denominator comes from an appended ones-column in V (out = [av | sum(p)]),
and the per-row normalization happens token-major where it is a cheap
per-partition scalar multiply. The two heads of a pair occupy partition
halves 0-63 / 64-127; their score matmuls are issued adjacently so the PE
runs them concurrently on disjoint row-groups (tile_position row tiling).

PSUM discipline: start=True marks the whole 2KB zero-region pending, so
every concurrently-accumulating group needs its own bank. The two PV
accumulators (per head) process the Left query block first, then reuse the
same banks for the Right block (the exp'd probability tiles persist in SBUF).

Matmul inputs are bf16 (fp32 PSUM accumulation); residual/layernorm fp32.
"""

import sys
from contextlib import ExitStack

import numpy as np

for _p in ("/opt/trn_rl_repo",):
    if _p not in sys.path:
        sys.path.insert(0, _p)

import ml_dtypes

import concourse.bass as bass
import concourse.mybir as mybir
import concourse.tile as tile
from concourse import bacc
from concourse.bass_utils import run_bass_kernel_spmd
from concourse.masks import make_identity

BF16 = ml_dtypes.bfloat16
F32 = mybir.dt.float32
BF = mybir.dt.bfloat16
P = 128
AF = mybir.ActivationFunctionType
ALU = mybir.AluOpType


def chunks(total, size=512):
    return [(s, min(size, total - s)) for s in range(0, total, size)]


class Cfg:
    """Problem geometry. Defaults = the real problem; overridable for sims."""

    def __init__(self, S=2048, D=1024, H=16, DFF=4096):
        self.S = S
        self.D = D
        self.H = H
        self.DFF = DFF
        self.HD = 64
        assert self.H * self.HD == self.D
        self.NQ = S // 2
        self.NSLOT = S // 256
        self.KT = D // P
        self.MT_FF = DFF // P
        self.NQ_T = self.NQ // P
        self.S_T = S // P


def build_nc(cfg: Cfg):
    S, D, H, DFF = cfg.S, cfg.D, cfg.H, cfg.DFF
    NQ, NSLOT, KT, MT_FF = cfg.NQ, cfg.NSLOT, cfg.KT, cfg.MT_FF
    NQ_T, S_T = cfg.NQ_T, cfg.S_T
    NPAIR = H // 2

    nc = bacc.Bacc(None, target_bir_lowering=False, debug=False)

    def din(name, shape, d=BF):
        return nc.dram_tensor(name, shape, d, kind="ExternalInput").ap()

    xt = din("xt", [D, S])                 # x[b].T, full sequence
    xtq = din("xtq", [P, KT * NQ])         # own tokens, [p, kt*NQ+t] layout
    xq = din("xq", [NQ, D], F32)           # own tokens, token-major (residual)
    masks = din("masks", [4, P, 2 * P])    # multiplicative causal masks (bf16)
    wqkv = din("wqkv", [D, 3 * D])         # q-part pre-scaled by 1/8
    bqkv = din("bqkv", [P, 2 * KT], F32)   # q,k bias packed (col%128, col//128)
    bvrow = din("bvrow", [1, D], F32)      # v bias as a row
    wap = din("wap", [D, D])
    bap = din("bap", [1, D], F32)
    wfc = din("wfc", [D, DFF])
    bfc = din("bfc", [P, MT_FF], F32)
    wmp = din("wmp", [DFF, D])
    bmp = din("bmp", [1, D], F32)
    g1 = din("g1", [1, D], F32)
    g2 = din("g2", [1, D], F32)
    b2 = din("b2", [1, D], F32)
    out = nc.dram_tensor("out", [NQ, D], F32, kind="ExternalOutput").ap()

    wqkv3 = wqkv.rearrange("(kt p) c -> kt p c", p=P)
    wap3 = wap.rearrange("(kt p) c -> kt p c", p=P)
    wfc3 = wfc.rearrange("(kt p) c -> kt p c", p=P)
    wmp3 = wmp.rearrange("(kt p) c -> kt p c", p=P)
    xtq3 = xtq.rearrange("p (kt t) -> p kt t", kt=KT)

    def bcast_row(ap):  # [1, D] DRAM -> [P, D] broadcast AP
        return bass.AP(tensor=ap.tensor, offset=ap.offset, ap=[[0, P], [1, D]])

    NSUB = (D + 511) // 512
    SUB = D // NSUB
    assert SUB * NSUB == D and SUB <= 512

    def layer_norm(nc, pool, dst, src, g_row, b_row, eps_t, tag, rows=None):
        """dst[P, D] = g * (src - mean)/sqrt(var+eps) + b, rowwise over D.
        rows=False skips the g/b application entirely."""
        skip_rows = rows is False
        rows = nc.gpsimd if rows in (None, False) else rows
        stats = pool.tile([P, NSUB, 6], F32, tag=f"{tag}_st")
        for sub in range(NSUB):
            nc.vector.bn_stats(stats[:, sub, :], src[:, sub * SUB:(sub + 1) * SUB])
        mv = pool.tile([P, 2], F32, tag=f"{tag}_mv")
        nc.vector.bn_aggr(mv, stats)
        rstd = pool.tile([P, 1], F32, tag=f"{tag}_rs")
        nc.scalar.activation(out=rstd, in_=mv[:, 1:2], func=AF.Sqrt,
                             bias=eps_t, scale=1.0)
        nc.vector.reciprocal(rstd, rstd)
        nc.vector.tensor_scalar(
            out=dst, in0=src, scalar1=mv[:, 0:1], scalar2=rstd,
            op0=ALU.subtract, op1=ALU.mult)
        if not skip_rows:
            rows.tensor_mul(dst, dst, g_row)
            rows.tensor_add(dst, dst, b_row)

    with tile.TileContext(nc) as tc, ExitStack() as top:
        const = top.enter_context(tc.tile_pool(name="const", bufs=1))
        bqkvS = const.tile([P, 2 * KT], F32)
        nc.sync.dma_start(out=bqkvS, in_=bqkv)
        epsS = const.tile([P, 1], F32)
        nc.vector.memset(epsS, 1e-5)
        bfcS = const.tile([P, MT_FF], F32)
        nc.sync.dma_start(out=bfcS, in_=bfc)

        npool = top.enter_context(tc.tile_pool(name="npool", bufs=1))
        NHALF = max(1, NQ // 512)

        with ExitStack() as s_a:
            preh = s_a.enter_context(tc.tile_pool(name="pre_xt", bufs=1))
            aper = s_a.enter_context(tc.tile_pool(name="aper", bufs=1))
            aSB = [aper.tile([P, D], BF, name=f"aSB{j}", tag=f"aSB{j}")
                   for j in range(NQ_T)]
            maskS = aper.tile([P, 4, 2 * P], BF)
            nc.sync.dma_start(out=maskS, in_=masks.rearrange("m k q -> k m q"))

            with ExitStack() as s_qkv:
                qkvp = s_qkv.enter_context(tc.tile_pool(name="qkvper", bufs=1))
                vA = qkvp.tile([P, S_T, H, 65], BF)
                abp = s_qkv.enter_context(tc.tile_pool(name="abp", bufs=1))
                xtqS = abp.tile([P, KT, NQ], BF)
                qtp = s_qkv.enter_context(tc.tile_pool(name="qtp", bufs=2))
                ktp = s_qkv.enter_context(tc.tile_pool(name="ktp", bufs=2))
                wp = s_qkv.enter_context(tc.tile_pool(name="wp", bufs=2))
                wvp = s_qkv.enter_context(tc.tile_pool(name="wvp", bufs=1))
                ptp = s_qkv.enter_context(tc.tile_pool(name="ptp", bufs=8))
                pm = s_qkv.enter_context(tc.tile_pool(name="pm", bufs=4))
                pqk = s_qkv.enter_context(
                    tc.tile_pool(name="pqk", bufs=2, space="PSUM"))
                psp = s_qkv.enter_context(
                    tc.tile_pool(name="psp", bufs=2, space="PSUM"))
                pvp = s_qkv.enter_context(
                    tc.tile_pool(name="pvp", bufs=1, space="PSUM"))

                # ---- input staging ----
                xtS = preh.tile([P, KT, S], BF, tag="bigslot")
                for h0, hw in chunks(S, S // 2):
                    nc.gpsimd.dma_start(
                        out=xtS[:, :, h0:h0 + hw],
                        in_=xt.rearrange("(kt p) t -> p kt t", p=P)
                        [:, :, h0:h0 + hw])
                def xt_slice(kt, a, b):
                    return xtS[:, kt, a:b]

                # ---- v projection (full sequence, all heads) ----
                with ExitStack() as ph:
                    bvp = ph.enter_context(tc.tile_pool(name="bvp", bufs=1))
                    bvS = bvp.tile([P, D], F32)
                    nc.scalar.dma_start(out=bvS, in_=bcast_row(bvrow))
                    for c0, cw in chunks(D):
                        wv = wvp.tile([P, KT, 512], BF, tag="wv")
                        nc.sync.dma_start(
                            out=wv[:, :, :cw],
                            in_=wqkv3[:, :, 2 * D + c0:2 * D + c0 + cw]
                            .rearrange("kt p c -> p kt c"))
                        h0, nh = c0 // 64, cw // 64
                        for mt in range(S_T):
                            ps = pqk.tile([P, 512], F32, tag="qk")
                            for kt in range(KT):
                                nc.tensor.matmul(
                                    ps[:, :cw],
                                    lhsT=xt_slice(kt, mt * P, (mt + 1) * P),
                                    rhs=wv[:, kt, :cw],
                                    start=(kt == 0), stop=(kt == KT - 1))
                            nc.vector.tensor_add(
                                out=vA[:, mt, h0:h0 + nh, 0:64],
                                in0=ps[:, :cw]
                                .rearrange("p (h d) -> p h d", d=64),
                                in1=bvS[:, c0:c0 + cw]
                                .rearrange("p (h d) -> p h d", d=64))
                    nc.vector.memset(vA[:, :, :, 64:65], 1.0)

                for h0, hw in chunks(NQ, NQ // 2):
                    nc.sync.dma_start(out=xtqS[:, :, h0:h0 + hw],
                                      in_=xtq3[:, :, h0:h0 + hw])

                # ---- per head-pair: q/k projection then attention ----
                for hp in range(NPAIR):
                    wq = wp.tile([P, KT, P], BF, tag="wq")
                    nc.sync.dma_start(
                        out=wq, in_=wqkv3[:, :, hp * P:(hp + 1) * P]
                        .rearrange("kt p c -> p kt c"))
                    qTt = qtp.tile([P, NQ], BF, tag="qT")
                    for c0, w in chunks(NQ):
                        ps = pqk.tile([P, 512], F32, tag="qk")
                        for kt in range(KT):
                            nc.tensor.matmul(
                                ps[:, :w], lhsT=wq[:, kt, :],
                                rhs=xtqS[:, kt, c0:c0 + w],
                                start=(kt == 0), stop=(kt == KT - 1))
                        nc.vector.tensor_scalar_add(
                            out=qTt[:, c0:c0 + w], in0=ps[:, :w],
                            scalar1=bqkvS[:, hp:hp + 1])

                    wk = wp.tile([P, KT, P], BF, tag="wk")
                    nc.sync.dma_start(
                        out=wk, in_=wqkv3[:, :, D + hp * P:D + (hp + 1) * P]
                        .rearrange("kt p c -> p kt c"))
                    kTt = ktp.tile([P, S], BF, tag="kT")
                    for c0, w in chunks(S):
                        ps = pqk.tile([P, 512], F32, tag="qk")
                        for kt in range(KT):
                            nc.tensor.matmul(
                                ps[:, :w], lhsT=wk[:, kt, :],
                                rhs=xt_slice(kt, c0, c0 + w),
                                start=(kt == 0), stop=(kt == KT - 1))
                        nc.vector.tensor_scalar_add(
                            out=kTt[:, c0:c0 + w], in0=ps[:, :w],
                            scalar1=bqkvS[:, KT + hp:KT + hp + 1])

                    for g in range(NSLOT // 2):
                        nkt = 4 * g + 4
                        # Left query block: accumulate while scores stream
                        pavL = [pvp.tile([P, 65], F32, tag=f"pav{hh}",
                                         name=f"pavL{hh}") for hh in range(2)]
                        pts = []

                        def pv_left(lo, pt):
                            for hh in range(2):
                                h = 2 * hp + hh
                                for ki in range(2):
                                    kt = lo + ki
                                    if kt < nkt - 2:
                                        nc.tensor.matmul(
                                            pavL[hh],
                                            lhsT=pt[:, hh, ki, 0:P],
                                            rhs=vA[:, kt, h, :],
                                            start=(kt == 0),
                                            stop=(kt == nkt - 3))

                        pend = None
                        for sc in range(nkt // 2):
                            lo = 2 * sc
                            ps = psp.tile([P, 2, 2, 2 * P], F32, tag="ps")
                            for ki in range(2):
                                kt = lo + ki
                                for hh in range(2):
                                    pb = 64 * hh
                                    nc.tensor.matmul(
                                        ps[:, hh, ki, :],
                                        lhsT=kTt[pb:pb + 64,
                                                 kt * P:(kt + 1) * P],
                                        rhs=qTt[pb:pb + 64,
                                                g * 2 * P:(g + 1) * 2 * P],
                                        start=True, stop=True)
                            pt = ptp.tile([P, 2, 2, 2 * P], BF, tag="pt")
                            nc.scalar.activation(out=pt, in_=ps, func=AF.Exp)
                            if lo >= nkt - 4:  # masks cover last 4 k-tiles
                                m0 = lo - (nkt - 4)
                                for hh in range(2):
                                    nc.vector.tensor_mul(
                                        pt[:, hh, :, :], pt[:, hh, :, :],
                                        maskS[:, m0:m0 + 2, :])
                            pts.append((lo, pt))
                            if pend is not None:
                                pv_left(*pend)
                            pend = (lo, pt)
                        pv_left(*pend)
                        for hh in range(2):
                            h = 2 * hp + hh
                            rec = pm.tile([P, 1], F32, tag="rec")
                            nc.vector.reciprocal(rec, pavL[hh][:, 64:65])
                            nc.vector.tensor_scalar_mul(
                                out=aSB[2 * g][:, h * 64:(h + 1) * 64],
                                in0=pavL[hh][:, 0:64], scalar1=rec)
                        # Right query block: replay persisted pt tiles into
                        # the same two banks (generation WAR on the drain)
                        pavR = [pvp.tile([P, 65], F32, tag=f"pav{hh}",
                                         name=f"pavR{hh}") for hh in range(2)]
                        for lo, pt in pts:
                            for hh in range(2):
                                h = 2 * hp + hh
                                for ki in range(2):
                                    kt = lo + ki
                                    nc.tensor.matmul(
                                        pavR[hh],
                                        lhsT=pt[:, hh, ki, P:2 * P],
                                        rhs=vA[:, kt, h, :],
                                        start=(kt == 0),
                                        stop=(kt == nkt - 1))
                        for hh in range(2):
                            h = 2 * hp + hh
                            rec = pm.tile([P, 1], F32, tag="rec")
                            nc.vector.reciprocal(rec, pavR[hh][:, 64:65])
                            nc.vector.tensor_scalar_mul(
                                out=aSB[2 * g + 1][:, h * 64:(h + 1) * 64],
                                in0=pavR[hh][:, 0:64], scalar1=rec)

                # prefetch aproj weights into the (now dead) xtS slot; the
                # DMA transfers while the attention tail drains
                wapS = preh.tile([P, KT, D], BF, tag="bigslot", name="wapS")
                nc.sync.dma_start(
                    out=wapS, in_=wap3.rearrange("kt p c -> p kt c"))
            # per-hp tiles freed here

            # ---------- Phase C: aproj + residual + LN1 (+ transposes) ----
            with ExitStack() as ph:
                pcc = ph.enter_context(tc.tile_pool(name="pcc", bufs=1))
                ppc = ph.enter_context(
                    tc.tile_pool(name="pc_ps", bufs=3, space="PSUM"))
                pmc = ph.enter_context(tc.tile_pool(name="pc_m", bufs=3))
                ptr = ph.enter_context(
                    tc.tile_pool(name="pc_tr", bufs=2, space="PSUM"))

                idb = pcc.tile([P, P], BF)
                make_identity(nc, idb)
                aT = [pcc.tile([P, NQ], BF, name=f"aT{ck}", tag=f"aT{ck}")
                      for ck in range(KT)]
                n_sb = [npool.tile([P, D], BF, name=f"n{mt}", tag=f"n{mt}")
                        for mt in range(NQ_T)]
                nT = [[npool.tile([P, min(512, NQ)], BF, name=f"nT{ck}_{hf}",
                                  tag=f"nT{ck}_{hf}") for hf in range(NHALF)]
                      for ck in range(KT)]
                bapS = pmc.tile([P, D], F32, tag="bapS", bufs=1)
                nc.sync.dma_start(out=bapS, in_=bcast_row(bap))
                g1S = pmc.tile([P, D], F32, tag="g1S", bufs=1)
                nc.sync.dma_start(out=g1S, in_=bcast_row(g1))
                bmpS = pmc.tile([P, D], F32, tag="bmpS", bufs=1)
                nc.sync.dma_start(out=bmpS, in_=bcast_row(bmp))

                def transpose_a(g):
                    # aSB[2g], aSB[2g+1] (token-major) -> aT (feature-major)
                    for j in (2 * g, 2 * g + 1):
                        for ck in range(KT):
                            pst = ptr.tile([P, P], BF, tag="pst")
                            nc.tensor.transpose(
                                pst, aSB[j][:, ck * P:(ck + 1) * P], idb)
                            nc.vector.tensor_copy(
                                out=aT[ck][:, j * P:(j + 1) * P], in_=pst)

                xq_pref = {}

                def xq_tile(mt):
                    if mt < NQ_T and mt not in xq_pref:
                        t = pmc.tile([P, D], F32, tag="xqS", name=f"xq{mt}")
                        nc.sync.dma_start(
                            out=t, in_=xq[mt * P:(mt + 1) * P, :])
                        xq_pref[mt] = t

                xq_tile(0)
                xq_tile(1)
                for mt in range(NQ_T):
                    if mt % 2 == 0:
                        transpose_a(mt // 2)
                    xq_tile(mt + 2)
                    xqS = xq_pref.pop(mt)
                    for c0, w in chunks(D):
                        ps = ppc.tile([P, 512], F32, tag="ps")
                        for kt in range(KT):
                            nc.tensor.matmul(
                                ps[:, :w],
                                lhsT=aT[kt][:, mt * P:(mt + 1) * P],
                                rhs=wapS[:, kt, c0:c0 + w],
                                start=(kt == 0), stop=(kt == KT - 1))
                        sl = slice(c0, c0 + w)
                        nc.vector.scalar_tensor_tensor(
                            out=xqS[:, sl], in0=ps[:, :w], scalar=1.0,
                            in1=xqS[:, sl], op0=ALU.mult, op1=ALU.add)
                        nc.vector.tensor_add(xqS[:, sl], xqS[:, sl], bapS[:, sl])
                    layer_norm(nc, pmc, n_sb[mt], xqS, None, None,
                               epsS, "ln1", rows=False)

                mt_per_half = NQ_T // NHALF
                for mt in range(NQ_T):
                    hf, mo = mt // mt_per_half, mt % mt_per_half
                    for ck in range(KT):
                        pst = ptr.tile([P, P], BF, tag="pst")
                        nc.tensor.transpose(
                            pst, n_sb[mt][:, ck * P:(ck + 1) * P], idb)
                        nc.vector.tensor_copy(
                            out=nT[ck][hf][:, mo * P:(mo + 1) * P], in_=pst)
                # residual copy: n = g1*n0 + (b1 + b_mproj), off critical path
                for mt in range(NQ_T):
                    nc.vector.tensor_mul(n_sb[mt], n_sb[mt], g1S)
                    nc.vector.tensor_add(n_sb[mt], n_sb[mt], bmpS)
        # aSB/aT freed here

        # -------- Phase D: MLP + residual + LN2 --------
        with ExitStack() as ph:
            pd = ph.enter_context(tc.tile_pool(name="pd", bufs=1))
            pdw = ph.enter_context(tc.tile_pool(name="pd_w", bufs=6))
            pwm = ph.enter_context(tc.tile_pool(name="pd_wm", bufs=6))
            pmd = ph.enter_context(tc.tile_pool(name="pd_m", bufs=2))
            # one PSUM pool for all of phase D (no mid-phase pool barrier):
            # fc uses tag "ps" (2 banks); mproj alternates tags mpA/mpB
            # (2 banks each, one live accumulation group per bank)
            ppd = ph.enter_context(
                tc.tile_pool(name="pd_ps", bufs=1, space="PSUM"))

            g2S = pd.tile([P, D], F32)
            nc.gpsimd.dma_start(out=g2S, in_=bcast_row(g2))
            b2S = pd.tile([P, D], F32)
            nc.gpsimd.dma_start(out=b2S, in_=bcast_row(b2))

            # fc: all tokens in one pass (weights streamed once)
            hT = pd.tile([P, MT_FF, NQ], BF)
            for m in range(MT_FF):
                wfcT = pdw.tile([P, KT, P], BF, tag="wfc")
                nc.sync.dma_start(
                    out=wfcT, in_=wfc3[:, :, m * P:(m + 1) * P]
                    .rearrange("kt p c -> p kt c"))
                for hf in range(NHALF):
                    ps = ppd.tile([P, 512], F32, tag="ps", bufs=2)
                    for kt in range(KT):
                        nc.tensor.matmul(
                            ps, lhsT=wfcT[:, kt, :], rhs=nT[kt][hf],
                            start=(kt == 0), stop=(kt == KT - 1))
                    nc.scalar.activation(
                        out=hT[:, m, hf * 512:(hf + 1) * 512], in_=ps,
                        func=AF.Gelu_apprx_tanh, bias=bfcS[:, m:m + 1],
                        scale=1.0)

            # mproj in two token-halves so LN2/output of half 0 overlaps
            # half 1's matmuls. Each 512-col chunk accumulates 4 token tiles
            # at once (sharing one wm stream) across two 2-bank psum tiles
            # whose [:, ml, :] regions are bank-aligned (one group per bank).
            MH = 2
            mts_per_h = NQ_T // MH
            for mh in range(MH):
                mts = list(range(mh * mts_per_h, (mh + 1) * mts_per_h))
                r2s = {mt: pmd.tile([P, D], F32, tag=f"r2_{mt % 4}",
                                    name=f"r2_{mt}")
                       for mt in mts}
                for ci, (c0, w) in enumerate(chunks(D)):
                    pss = [ppd.tile([P, 2, 512], F32, tag=f"mp{half}",
                                    bufs=1, name=f"pss{mh}_{ci}_{half}")
                           for half in range(2)]
                    for kt in range(MT_FF):
                        wmT = pwm.tile([P, 512], BF, tag="wm")
                        nc.gpsimd.dma_start(out=wmT[:, :w],
                                            in_=wmp3[kt][:, c0:c0 + w])
                        for ml, mt in enumerate(mts):
                            nc.tensor.matmul(
                                pss[ml // 2][:, ml % 2, :w],
                                lhsT=hT[:, kt, mt * P:(mt + 1) * P],
                                rhs=wmT[:, :w],
                                start=(kt == 0), stop=(kt == MT_FF - 1))
                    sl = slice(c0, c0 + w)
                    for ml, mt in enumerate(mts):
                        nc.vector.scalar_tensor_tensor(
                            out=r2s[mt][:, sl], in0=pss[ml // 2][:, ml % 2, :w],
                            scalar=1.0, in1=n_sb[mt][:, sl],
                            op0=ALU.mult, op1=ALU.add)
                for mt in mts:
                    oS = pmd.tile([P, D], F32, tag="oS")
                    layer_norm(nc, pmd, oS, r2s[mt], g2S, b2S, epsS,
                               "ln2", rows=nc.vector)
                    nc.scalar.dma_start(
                        out=out[mt * P:(mt + 1) * P, :], in_=oS)

    nc.finalize()
    return nc


def host_inputs(cfg: Cfg, x, w_attn, b_attn, w_aproj, b_aproj, g1, b1,
                w_fc, b_fc, w_mproj, b_mproj, g2, b2, n_cores=8):
    """Build the per-core input maps + output scatter indices."""
    S, D = cfg.S, cfg.D
    NSLOT, KT, MT_FF = cfg.NSLOT, cfg.KT, cfg.MT_FF
    NQ = cfg.NQ

    wqkv = np.ascontiguousarray(w_attn, np.float32).copy()
    wqkv[:, :D] *= 0.125
    wqkv = wqkv.astype(BF16)
    b_adj = np.asarray(b_attn, np.float32).copy()
    b_adj[:D] *= 0.125
    bqkv = np.ascontiguousarray(b_adj[:2 * D].reshape(2 * KT, P).T, np.float32)
    shared = dict(
        wqkv=wqkv,
        bqkv=bqkv,
        bvrow=np.ascontiguousarray(
            np.asarray(b_attn, np.float32)[2 * D:].reshape(1, D)),
        wap=np.asarray(w_aproj).astype(BF16),
        bap=np.asarray(b_aproj, np.float32).reshape(1, D),
        wfc=(np.asarray(w_fc, np.float32)
             * np.asarray(g1, np.float32)[:, None]).astype(BF16),
        bfc=np.ascontiguousarray(
            (np.asarray(b_fc, np.float32)
             + np.asarray(b1, np.float32) @ np.asarray(w_fc, np.float32))
            .reshape(MT_FF, P).T),
        wmp=np.asarray(w_mproj).astype(BF16),
        bmp=(np.asarray(b_mproj, np.float32)
             + np.asarray(b1, np.float32)).reshape(1, D),
        g1=np.asarray(g1, np.float32).reshape(1, D),
        g2=np.asarray(g2, np.float32).reshape(1, D),
        b2=np.asarray(b2, np.float32).reshape(1, D),
    )

    tri_kq = (np.arange(P)[:, None] <= np.arange(P)[None, :]).astype(np.float32)
    ones = np.ones((P, P), np.float32)
    zeros = np.zeros((P, P), np.float32)

    def m4(r):
        if r == 0:
            rows = [[tri_kq, ones], [zeros, ones], [zeros, tri_kq],
                    [zeros, zeros]]
        else:
            rows = [[ones, ones], [tri_kq, ones], [zeros, ones],
                    [zeros, tri_kq]]
        return np.stack([np.concatenate(rr, axis=1) for rr in rows])

    m_even, m_odd = m4(0), m4(1)

    in_maps = []
    idx_all = []
    for c in range(n_cores):
        b, r = c // 2, c % 2
        idx = (np.arange(NSLOT)[:, None] * 256 + 128 * r
               + np.arange(P)[None, :]).ravel()
        idx_all.append((b, idx))
        xb = np.asarray(x[b], np.float32)
        m = dict(shared)
        m["xt"] = np.ascontiguousarray(xb.T).astype(BF16)
        xtq_km = np.ascontiguousarray(xb[idx]).T.reshape(KT, P, NQ)
        m["xtq"] = np.ascontiguousarray(
            xtq_km.transpose(1, 0, 2).reshape(P, KT * NQ)).astype(BF16)
        m["xq"] = np.ascontiguousarray(xb[idx], np.float32)
        m["masks"] = (m_even if r == 0 else m_odd).astype(BF16)
        in_maps.append(m)
    return in_maps, idx_all


_CACHE = {}


def _get_nc(cfg: Cfg):
    key = (cfg.S, cfg.D, cfg.H, cfg.DFF)
    if key not in _CACHE:
        _CACHE[key] = build_nc(cfg)
    return _CACHE[key]


def kernel(x, w_attn, b_attn, w_aproj, b_aproj, g1, b1, w_fc, b_fc,
           w_mproj, b_mproj, g2, b2):
    cfg = Cfg()
    nc = _get_nc(cfg)
    in_maps, idx_all = host_inputs(cfg, x, w_attn, b_attn, w_aproj, b_aproj,
                                   g1, b1, w_fc, b_fc, w_mproj, b_mproj,
                                   g2, b2)
    res = run_bass_kernel_spmd(nc, in_maps, list(range(8)))
    B = x.shape[0]
    y = np.empty((B, cfg.S, cfg.D), np.float32)
    for c in range(8):
        b, idx = idx_all[c]
        y[b][idx] = res.results[c]["out"]
    return y
